# revision 24
# baseline (speedup 1.0000x reference)
"""AnomalyAwareMemory Trainium2 kernel (8 NeuronCores, single SPMD NEFF).

v3 strategy:

* Projection folding: bk cancels in softmax, so scores = SC*(z@(Wq^T Wk))@mem^T
  -- no K projection.  num = Wv^T(mem^T e) + bv*den, so no V projection either.
* Host-packed partition-major inputs ([128, big] arrays, one large DMA
  descriptor per partition) -- kills the ~35us descriptor-bound load phase.
* X = 2I - A instead of Newton-Schulz: A = 0.99 I + 0.01 cov + eps is within
  ~8e-3 of I, so inv(A) = 2I - A + O(|E|^2 ~ 6e-5), below the fp16 noise the
  baseline's 3 NS iterations bottom out at.
* No [128,*] sqrt: top-16 / crossing run on squared distances (monotone);
  the weight list is transformed into squared space instead.  mu comes free
  from a ones-column on the z^T z stats matmul.  Mahalanobis row-dots via
  scalar_tensor_tensor with accum_out (one op per tile).
* Eviction as in the baseline: crossing count R of sorted importance vs
  sorted weights, value thresholds, exp-bias masking (evicted slots /
  non-inserted pseudo-keys get exp bias -(1e4+20) -> exact 0 in bf16).
* Memory-sharded flash attention, 512 queries per chunk, q-major [128,257]
  partials (Wv applied per chunk with numW-chunks as lhsT), den accumulated
  on vector+gpsimd.  Two ReduceScatters ([1024,257] f32): RS-A after chunk 1
  overlaps chunks 2-3; only RS-B (~19us) is exposed.  Finalize = reciprocal
  + one fused (num*rec + z + 0.5 bv) op, emitted under tile_wait_until so
  collective-gated ops land after all attention work in the engine queues.
"""

import ml_dtypes
import numpy as np

import concourse.bass as bass
import concourse.mybir as mybir
from concourse import bacc
from concourse.tile import TileContext
from concourse.masks import make_identity
from concourse.bass_utils import run_bass_kernel_spmd

f32 = mybir.dt.float32
f16 = mybir.dt.float16
bf16 = mybir.dt.bfloat16
i32 = mybir.dt.int32
AF = mybir.ActivationFunctionType
ALU = mybir.AluOpType
AX = mybir.AxisListType

N = 2048          # batch
D = 256           # embedding dim
MEM = 16384       # memory slots
NC = 8            # cores
JL = MEM // NC    # 2048 memory slots per core
QL = N // NC      # 256 output rows per core (2 chunks of 128)
NT = N // 128     # 16 z tiles
JT = JL // 128    # 16 local memory tiles
KT_Z = QL // 128  # 2 local z pseudo-key tiles
NJT = JT + KT_Z   # 18 flash tiles
B = 16            # top-B merge width
SHIFT = 20.0      # global exp shift, cancels in num/den
SC = 1.0 / (16.0 * 0.1)   # 1/(sqrt(D)*TEMP)
MOM = 0.01
BIG = 1e30
BIGM = 1e4
ZW = NT * (D + 1)         # packed z columns (ones col per tile)
# aux pack layout (f32 [128, AUXW])
AUX_MW = 0                # [128, 128] memory weights
AUX_WLOC = 128            # [128, JT] local weights col-per-tile
AUX_RCOV = 144            # [128, 512] running_cov (2 chunks)
AUX_BQ = 656              # 2 cols: bq as columns
AUX_RM = 658              # 2 cols: running_mean as columns
AUXW = 660


def build(debug: bool = False) -> bacc.Bacc:
    nc = bacc.Bacc(num_devices=NC)

    zp_ext = nc.declare_dram_parameter("zp", [128, ZW], f16, isOutput=False)
    ztp_ext = nc.declare_dram_parameter("ztp", [128, 2 * N], f16, isOutput=False)
    mtp_ext = nc.declare_dram_parameter("mtp", [128, 2 * JL], f16, isOutput=False)
    mbp_ext = nc.declare_dram_parameter("mbp", [128, JT * D], bf16, isOutput=False)
    zktp_ext = nc.declare_dram_parameter("zktp", [128, 2 * QL], f16, isOutput=False)
    zkbp_ext = nc.declare_dram_parameter("zkbp", [128, KT_Z * D], bf16, isOutput=False)
    zkf_ext = nc.declare_dram_parameter("zkf", [128, KT_Z * D], f32, isOutput=False)
    wvtp_ext = nc.declare_dram_parameter("wvtp", [128, 2 * D], bf16, isOutput=False)
    wqp_ext = nc.declare_dram_parameter("wqp", [128, 512], f16, isOutput=False)
    wkp_ext = nc.declare_dram_parameter("wkp", [128, 512], f16, isOutput=False)
    aux_ext = nc.declare_dram_parameter("aux", [128, AUXW], f32, isOutput=False)
    lab_ext = nc.declare_dram_parameter("labels", [1, N], i32, isOutput=False)
    rmr_ext = nc.declare_dram_parameter("rmrow", [1, D], f32, isOutput=False)
    bvr_ext = nc.declare_dram_parameter("bvrow", [1, D], f32, isOutput=False)
    out_ext = nc.declare_dram_parameter("out", [QL, D], f32, isOutput=True)
    dbg = {}
    if debug:
        for nm, shp in [("dbg_S", [128, D]), ("dbg_X", [128, D]),
                        ("dbg_qq", [128, NT]), ("dbg_qq16", [1, B]),
                        ("dbg_w16", [1, B]), ("dbg_thw", [1, 2]),
                        ("dbg_keep", [128, JT]), ("dbg_ins", [128, KT_Z]),
                        ("dbg_QWT", [128, 512]), ("dbg_mu", [1, D]),
                        ("dbg_ab", [1, 8]), ("dbg_W2", [128, D])]:
            dbg[nm] = nc.declare_dram_parameter(nm, shp, f32, isOutput=True)

    with TileContext(nc) as tc:
        with (
            tc.tile_pool(name="per", bufs=1) as per,          # persistent sbuf
            tc.tile_pool(name="wrk", bufs=4) as wrk,          # rotating sbuf
            tc.tile_pool(name="dram", bufs=1, space="DRAM") as dram,
        ):
            # phase-scoped PSUM pools (closed before attention pools open)
            pre_ctx = tc.tile_pool(name="pre_ps", bufs=3, space="PSUM")
            pre = pre_ctx.__enter__()
            ptr_ctx = tc.tile_pool(name="ptr", bufs=2, space="PSUM")
            ptr = ptr_ctx.__enter__()
            prj_ctx = tc.tile_pool(name="prj_ps", bufs=2, space="PSUM")
            prj = prj_ctx.__enter__()
            qqp_ctx = tc.tile_pool(name="qq_ps", bufs=1, space="PSUM")
            qqp = qqp_ctx.__enter__()

            # ---------------- input DMAs (z first: stats gate phase A) ------
            zbig = per.tile([128, ZW], f16, tag="zbig")
            for i in range(8):
                w = ZW // 8
                nc.sync.dma_start(out=zbig[:, i * w:(i + 1) * w],
                                  in_=zp_ext[:, i * w:(i + 1) * w])
            z16 = [zbig[:, t * (D + 1):(t + 1) * (D + 1)] for t in range(NT)]

            labi = per.tile([1, N], i32, tag="labi")
            nc.sync.dma_start(out=labi, in_=lab_ext[:, :])
            aux = per.tile([128, AUXW], f32, tag="aux")
            nc.sync.dma_start(out=aux, in_=aux_ext[:, :])
            rmrow = per.tile([1, D], f32, tag="rmrow")
            nc.sync.dma_start(out=rmrow, in_=rmr_ext[:, :])
            bvrow = per.tile([1, D], f32, tag="bvrow")
            nc.sync.dma_start(out=bvrow, in_=bvr_ext[:, :])

            ztb = per.tile([128, 2 * N], f16, tag="ztb")
            for i in range(4):
                w = 2 * N // 4
                nc.sync.dma_start(out=ztb[:, i * w:(i + 1) * w],
                                  in_=ztp_ext[:, i * w:(i + 1) * w])
            zT = [ztb[:, c * N:(c + 1) * N] for c in range(2)]

            wqb = per.tile([128, 512], f16, tag="wqb")
            nc.gpsimd.dma_start(out=wqb, in_=wqp_ext[:, :])
            wkb = per.tile([128, 512], f16, tag="wkb")
            nc.gpsimd.dma_start(out=wkb, in_=wkp_ext[:, :])
            wvtb = per.tile([128, 2 * D], bf16, tag="wvtb")
            nc.gpsimd.dma_start(out=wvtb, in_=wvtp_ext[:, :])
            wvT = [wvtb[:, c * D:(c + 1) * D] for c in range(2)]
            wq16 = [wqb[:, c * D:(c + 1) * D] for c in range(2)]
            wk16 = [wkb[:, c * D:(c + 1) * D] for c in range(2)]

            zktb = per.tile([128, 2 * QL], f16, tag="zktb")
            nc.gpsimd.dma_start(out=zktb, in_=zktp_ext[:, :])
            zkT = [zktb[:, c * QL:(c + 1) * QL] for c in range(2)]
            zkbb = per.tile([128, KT_Z * D], bf16, tag="zkbb")
            nc.gpsimd.dma_start(out=zkbb, in_=zkbp_ext[:, :])
            zk16b = [zkbb[:, t * D:(t + 1) * D] for t in range(KT_Z)]
            zk32 = per.tile([128, KT_Z * D], f32, tag="zk32")
            nc.scalar.dma_start(out=zk32, in_=zkf_ext[:, :])

            mtb = per.tile([128, 2 * JL], f16, tag="mtb")
            for i in range(4):
                w = 2 * JL // 4
                nc.scalar.dma_start(out=mtb[:, i * w:(i + 1) * w],
                                    in_=mtp_ext[:, i * w:(i + 1) * w])
            memT = [mtb[:, c * JL:(c + 1) * JL] for c in range(2)]
            mbb = per.tile([128, JT * D], bf16, tag="mbb")
            for i in range(4):
                w = JT * D // 4
                nc.scalar.dma_start(out=mbb[:, i * w:(i + 1) * w],
                                    in_=mbp_ext[:, i * w:(i + 1) * w])
            mem16b = [mbb[:, t * D:(t + 1) * D] for t in range(JT)]

            wfull = aux[:, AUX_MW:AUX_MW + 128]
            wloc = aux[:, AUX_WLOC:AUX_WLOC + JT]
            rcov_s = []
            for c in range(2):
                t = per.tile([128, D], f32, tag=f"rcov_{c}")
                nc.scalar.mul(out=t, in_=aux[:, AUX_RCOV + c * D:AUX_RCOV + (c + 1) * D],
                              mul=1.0 - MOM)
                rcov_s.append(t)
            bqcol16 = []
            for c in range(2):
                t = per.tile([128, 1], f16, tag=f"bqcol16_{c}")
                nc.scalar.copy(out=t, in_=aux[:, AUX_BQ + c:AUX_BQ + c + 1])
                bqcol16.append(t)

            # ---------------- constants ----------------
            ident32 = per.tile([128, 128], f32, tag="ident32")
            make_identity(nc, ident32)
            ones11 = per.tile([1, 1], f32, tag="ones11")
            nc.vector.memset(ones11, 1.0)
            onecol32 = per.tile([128, 1], f32, tag="onecol32")
            nc.vector.memset(onecol32, 1.0)
            I2 = []     # 2*I (f16) row chunk c
            for c in range(2):
                t2 = per.tile([128, D], f16, tag=f"I2_{c}")
                nc.gpsimd.memset(t2, 0.0)
                nc.gpsimd.affine_select(out=t2, in_=t2, compare_op=ALU.not_equal,
                                        fill=2.0, base=128 * c,
                                        pattern=[[-1, D]], channel_multiplier=1)
                I2.append(t2)

            # residual rows + 0.5*bv, one [128, D] tile per output half
            bvrep = per.tile([128, D], f32, tag="bvrep")
            nc.gpsimd.partition_broadcast(bvrep, bvrow)
            halfbv = per.tile([128, D], f32, tag="halfbv")
            nc.scalar.mul(out=halfbv, in_=bvrep, mul=0.5)
            zkadj = []
            for h in range(KT_Z):
                t = per.tile([128, D], f32, tag=f"zkadj_{h}")
                nc.vector.tensor_tensor(out=t, in0=zk32[:, h * D:(h + 1) * D],
                                         in1=halfbv, op=ALU.add)
                zkadj.append(t)

            # ---------------- W2 = Wq^T @ Wk;  bqwk = bq @ Wk ----------------
            W2 = []
            for dm in range(2):
                ps = prj.tile([128, D], f32, tag="acc")
                for kc in range(2):
                    nc.tensor.matmul(ps, wq16[kc][:, dm * 128:(dm + 1) * 128],
                                     wk16[kc], start=(kc == 0), stop=(kc == 1))
                t = per.tile([128, D], f16, tag=f"W2_{dm}")
                nc.scalar.copy(out=t, in_=ps)
                W2.append(t)
            if debug:
                dw2 = per.tile([128, D], f32, tag="dw2")
                nc.vector.tensor_copy(out=dw2, in_=W2[0])
                nc.sync.dma_start(out=dbg["dbg_W2"][:, :], in_=dw2)
            ps_bq = pre.tile([1, D], f32, tag="acc")
            for kc in range(2):
                nc.tensor.matmul(ps_bq, bqcol16[kc], wk16[kc],
                                 start=(kc == 0), stop=(kc == 1))
            bqwk_row = per.tile([1, D], f32, tag="bqwk_row")
            nc.vector.tensor_scalar(out=bqwk_row, in0=ps_bq, scalar1=SC,
                                    scalar2=None, op0=ALU.mult)
            bqwk_col = []
            for c in range(2):
                p = ptr.tile([128, 1], f32, tag="tr")
                nc.tensor.matmul(p, bqwk_row[0:1, c * 128:(c + 1) * 128], ones11,
                                 start=True, stop=True)
                t = per.tile([128, 1], f32, tag=f"bqwk_col_{c}")
                nc.vector.tensor_copy(out=t, in_=p)
                bqwk_col.append(t)

            # ---------------- QWT = SC * (W2^T z^T + bqwk^T) ----------------
            QWT = [per.tile([128, N], f16, tag=f"QWT_{c}", name=f"QWT_{c}") for c in range(2)]
            for dm in range(2):
                for qc in range(N // 512):
                    ps = prj.tile([128, 512], f32, tag="acc")
                    for dc in range(2):
                        nc.tensor.matmul(ps, W2[dc][:, dm * 128:(dm + 1) * 128],
                                         zT[dc][:, qc * 512:(qc + 1) * 512],
                                         start=(dc == 0), stop=(dc == 1))
                    nc.scalar.activation(out=QWT[dm][:, qc * 512:(qc + 1) * 512],
                                         in_=ps, func=AF.Identity,
                                         bias=bqwk_col[dm], scale=SC)
            if debug:
                dq = per.tile([128, 512], f32, tag="dqw")
                nc.vector.tensor_copy(out=dq, in_=QWT[0][:, 0:512])
                nc.sync.dma_start(out=dbg["dbg_QWT"][:, :], in_=dq)

            # ---------------- phase A: stats -> thresholds -> exp biases ----
            with tc.high_priority():
                # KL(label dist || uniform)
                sc2 = per.tile([1, 8], f32, tag="sc2")  # [dmin dmax rden kl a b 1/a _]
                labf = per.tile([1, N], f32, tag="labf")
                nc.vector.tensor_copy(out=labf, in_=labi)
                cnt1 = per.tile([1, 1], f32, tag="cnt1")
                nc.vector.tensor_reduce(out=cnt1, in_=labf, axis=AX.X, op=ALU.add)
                pvec = per.tile([1, 2], f32, tag="pvec")
                nc.vector.tensor_scalar(out=pvec[:, 1:2], in0=cnt1, scalar1=1.0 / N,
                                        scalar2=None, op0=ALU.mult)
                nc.vector.tensor_scalar(out=pvec[:, 0:1], in0=pvec[:, 1:2],
                                        scalar1=-1.0, scalar2=1.0,
                                        op0=ALU.mult, op1=ALU.add)
                lnin = per.tile([1, 2], f32, tag="lnin")
                nc.vector.tensor_scalar(out=lnin, in0=pvec, scalar1=2.0, scalar2=1e-8,
                                        op0=ALU.mult, op1=ALU.max)
                lnv = per.tile([1, 2], f32, tag="lnv")
                nc.scalar.activation(out=lnv, in_=lnin, func=AF.Ln)
                terms = per.tile([1, 2], f32, tag="terms")
                nc.vector.tensor_mul(terms, pvec, lnv)
                klr = per.tile([1, 1], f32, tag="klr")
                nc.vector.tensor_reduce(out=klr, in_=terms, axis=AX.X, op=ALU.add)
                nc.vector.tensor_scalar(out=sc2[:, 3:4], in0=klr, scalar1=0.0,
                                        scalar2=None, op0=ALU.max)

                def top16_stage(cur, tag, pdim):
                    tb = per.tile([pdim, B], f32, tag=tag)
                    for r in range(2):
                        nc.vector.max(out=tb[:, r * 8:(r + 1) * 8], in_=cur)
                        nc.vector.match_replace(out=cur, in_to_replace=tb[:, r * 8:(r + 1) * 8],
                                                in_values=cur, imm_value=-BIG)
                    return tb

                def flatten_16x16(tb, tag):
                    db = dram.tile([B, B], f32, tag=f"{tag}_d")
                    nc.sync.dma_start(out=db, in_=tb)
                    flat = per.tile([1, B * B], f32, tag=f"{tag}_f")
                    nc.sync.dma_start(
                        out=flat,
                        in_=db.rearrange("p f -> (p f)").rearrange(
                            "(a b) -> a b", a=1))
                    return flat

                def global_top16(src128, tag):
                    t1 = top16_stage(src128, f"{tag}_t1", 128)      # [128, 16]
                    pT = ptr.tile([B, 128], f32, tag="tr")
                    nc.tensor.transpose(pT, t1, ident32)
                    t1t = per.tile([B, 128], f32, tag=f"{tag}_tt")
                    nc.vector.tensor_copy(out=t1t, in_=pT)
                    t2 = top16_stage(t1t, f"{tag}_t2", B)           # [16, 16]
                    flat = flatten_16x16(t2, tag)                   # [1, 256]
                    return top16_stage(flat, f"{tag}_t3", 1)        # [1, 16]

                # weights bottom-16 (ascending): independent, runs off aux
                wneg = per.tile([128, 128], f32, tag="wneg")
                nc.vector.tensor_scalar(out=wneg, in0=wfull, scalar1=-1.0,
                                        scalar2=None, op0=ALU.mult)
                w16neg = global_top16(wneg, "wtop")
                w16v = per.tile([1, B], f32, tag="w16v")
                nc.vector.tensor_scalar(out=w16v, in0=w16neg, scalar1=-1.0,
                                        scalar2=None, op0=ALU.mult)

                # ---- mu first (gates rmcol/cT), then S = z^T z ----
                onecol16 = per.tile([128, 1], f16, tag="onecol16")
                nc.vector.memset(onecol16, 1.0)
                pmu = pre.tile([1, D + 1], f32, tag="acc")
                for t in range(NT):
                    nc.tensor.matmul(pmu, onecol16, z16[t],
                                     start=(t == 0), stop=(t == NT - 1))
                murow = per.tile([1, D], f32, tag="murow")
                nc.vector.tensor_scalar(out=murow, in0=pmu[0:1, 0:D],
                                        scalar1=1.0 / N, scalar2=None,
                                        op0=ALU.mult)
                mucol = []
                for c in range(2):
                    p = ptr.tile([128, 1], f32, tag="tr")
                    nc.tensor.matmul(p, murow[0:1, c * 128:(c + 1) * 128], ones11,
                                     start=True, stop=True)
                    t = per.tile([128, 1], f32, tag=f"mucol_{c}")
                    nc.vector.tensor_copy(out=t, in_=p)
                    mucol.append(t)
                S_sb = []
                for mc in range(2):
                    ps = pre.tile([128, D], f32, tag="acc")
                    for t in range(NT):
                        nc.tensor.matmul(ps, z16[t][:, mc * 128:(mc + 1) * 128],
                                         z16[t][:, 0:D],
                                         start=(t == 0), stop=(t == NT - 1))
                    sb = per.tile([128, D], f32, tag=f"S_{mc}")
                    nc.vector.tensor_scalar(out=sb, in0=ps,
                                            scalar1=MOM / (N - 1),
                                            scalar2=None, op0=ALU.mult)
                    S_sb.append(sb)
                if debug:
                    ds = per.tile([128, D], f32, tag="ds")
                    nc.vector.tensor_copy(out=ds, in_=S_sb[0])
                    nc.sync.dma_start(out=dbg["dbg_S"][:, :], in_=ds)

                mu16 = per.tile([1, D], f16, tag="mu16")
                nc.scalar.copy(out=mu16, in_=murow)
                if debug:
                    nc.sync.dma_start(out=dbg["dbg_mu"][:, :], in_=murow)

                # rm row / cols / broadcast
                rm = per.tile([1, D], f32, tag="rm")
                nc.vector.tensor_scalar(out=rm, in0=rmrow, scalar1=1.0 - MOM,
                                        scalar2=None, op0=ALU.mult)
                musc = per.tile([1, D], f32, tag="musc")
                nc.vector.tensor_scalar(out=musc, in0=murow, scalar1=MOM,
                                        scalar2=None, op0=ALU.mult)
                nc.vector.tensor_add(rm, rm, musc)
                rmcol = []
                for c in range(2):
                    t = per.tile([128, 1], f32, tag=f"rmcol_{c}")
                    nc.vector.tensor_scalar(
                        out=t, in0=aux[:, AUX_RM + c:AUX_RM + c + 1],
                        scalar1=1.0 - MOM, scalar2=None, op0=ALU.mult)
                    t2 = per.tile([128, 1], f32, tag=f"rmcol2_{c}")
                    nc.vector.tensor_scalar(out=t2, in0=mucol[c], scalar1=MOM,
                                            scalar2=None, op0=ALU.mult)
                    nc.vector.tensor_add(t, t, t2)
                    rmcol.append(t)

                # ---- X = 2I - A,  A = (1-mom)*rcov + mom*cov ----
                X = []
                for mc in range(2):
                    pmo = pre.tile([128, D], f32, tag="acc")
                    nc.tensor.matmul(pmo, mu16[:, mc * 128:(mc + 1) * 128], mu16,
                                     start=True, stop=True)
                    acc = per.tile([128, D], f32, tag=f"A32_{mc}")
                    nc.vector.tensor_add(acc, S_sb[mc], rcov_s[mc])
                    nc.vector.scalar_tensor_tensor(
                        out=acc, in0=pmo, scalar=-MOM * N / (N - 1), in1=acc,
                        op0=ALU.mult, op1=ALU.add)
                    x = per.tile([128, D], f16, tag=f"X_{mc}")
                    nc.vector.tensor_tensor(out=x, in0=I2[mc], in1=acc,
                                            op=ALU.subtract)
                    X.append(x)
                if debug:
                    dx = per.tile([128, D], f32, tag="dx")
                    nc.vector.tensor_copy(out=dx, in_=X[0])
                    nc.sync.dma_start(out=dbg["dbg_X"][:, :], in_=dx)

                # ---- Mahalanobis squared distances (all N) ----
                rmcol16 = []
                for c in range(2):
                    t = per.tile([128, 1], f16, tag=f"rmcol16_{c}")
                    nc.vector.tensor_copy(out=t, in_=rmcol[c])
                    rmcol16.append(t)
                cT = [per.tile([128, N], f16, tag=f"cT_{c}", name=f"cT_{c}") for c in range(2)]
                for c in range(2):
                    for hh in range(2):
                        nc.vector.tensor_tensor(
                            out=cT[c][:, hh * 1024:(hh + 1) * 1024],
                            in0=zT[c][:, hh * 1024:(hh + 1) * 1024],
                            in1=rmcol16[c].to_broadcast([128, 1024]),
                            op=ALU.subtract)
                # X symmetric: qq[n] = sum_d cT[d,n] * (X cT)[d,n], summed on PE
                XcT = [per.tile([128, N], f16, tag=f"XcT_{c}", name=f"XcT_{c}")
                       for c in range(2)]
                for dm in range(2):
                    for ns in range(4):
                        pX = pre.tile([128, 512], f32, tag="acc")
                        for dc in range(2):
                            nc.tensor.matmul(pX, X[dc][:, dm * 128:(dm + 1) * 128],
                                             cT[dc][:, ns * 512:(ns + 1) * 512],
                                             start=(dc == 0), stop=(dc == 1))
                        dst = XcT[dm][:, ns * 512:(ns + 1) * 512]
                        if ns % 2 == 0:
                            nc.scalar.copy(out=dst, in_=pX)
                        else:
                            nc.vector.tensor_copy(out=dst, in_=pX)
                Y = [per.tile([128, N], f16, tag=f"Y_{c}", name=f"Y_{c}")
                     for c in range(2)]
                for c in range(2):
                    for hh in range(2):
                        nc.vector.tensor_tensor(
                            out=Y[c][:, hh * 1024:(hh + 1) * 1024],
                            in0=cT[c][:, hh * 1024:(hh + 1) * 1024],
                            in1=XcT[c][:, hh * 1024:(hh + 1) * 1024],
                            op=ALU.mult)
                qq_ps = qqp.tile([128, NT], f32, tag="qqps")
                for t in range(NT):
                    for dc in range(2):
                        nc.tensor.matmul(qq_ps[:, t:t + 1],
                                         Y[dc][:, t * 128:(t + 1) * 128], onecol16,
                                         start=(dc == 0), stop=(dc == 1))
                qq = per.tile([128, NT], f32, tag="qq")
                nc.vector.tensor_copy(out=qq, in_=qq_ps)
                nc.vector.tensor_scalar(out=qq, in0=qq, scalar1=1e-8, scalar2=None,
                                        op0=ALU.max)
                if debug:
                    nc.sync.dma_start(out=dbg["dbg_qq"][:, :], in_=qq)

                # dmin/dmax from squared extremes (single tiny sqrt)
                dmm = per.tile([128, 2], f32, tag="dmm")
                nc.vector.tensor_reduce(out=dmm[:, 0:1], in_=qq, axis=AX.X, op=ALU.min)
                nc.vector.tensor_reduce(out=dmm[:, 1:2], in_=qq, axis=AX.X, op=ALU.max)
                qex = per.tile([1, 2], f32, tag="qex")
                for k, op in ((0, ALU.min), (1, ALU.max)):
                    p = ptr.tile([1, 128], f32, tag="tr")
                    nc.tensor.transpose(p, dmm[:, k:k + 1], ident32)
                    row = per.tile([1, 128], f32, tag=f"drow_{k}")
                    nc.vector.tensor_copy(out=row, in_=p)
                    nc.vector.tensor_reduce(out=qex[:, k:k + 1], in_=row, axis=AX.X, op=op)
                nc.scalar.activation(out=sc2[:, 0:2], in_=qex, func=AF.Sqrt)
                # exp table warm-up, tied to the sqrt result so it runs here
                warm = per.tile([1, 1], f32, tag="warm")
                nc.scalar.activation(out=warm, in_=sc2[:, 0:1], func=AF.Exp,
                                     scale=0.0)

                # rden = 1/(dmax-dmin+1e-8); a = rden*kl; b = (1-dmin*rden)*kl
                dd = per.tile([1, 1], f32, tag="dd")
                nc.vector.tensor_sub(dd, sc2[:, 1:2], sc2[:, 0:1])
                nc.vector.tensor_scalar(out=dd, in0=dd, scalar1=1e-8, scalar2=None,
                                        op0=ALU.add)
                nc.vector.reciprocal(out=sc2[:, 2:3], in_=dd)
                nc.vector.tensor_mul(sc2[:, 4:5], sc2[:, 2:3], sc2[:, 3:4])
                t5 = per.tile([1, 1], f32, tag="t5")
                nc.vector.tensor_mul(t5, sc2[:, 0:1], sc2[:, 2:3])
                nc.vector.tensor_scalar(out=t5, in0=t5, scalar1=-1.0, scalar2=1.0,
                                        op0=ALU.mult, op1=ALU.add)
                nc.vector.tensor_mul(sc2[:, 5:6], t5, sc2[:, 3:4])
                nc.vector.reciprocal(out=sc2[:, 6:7], in_=sc2[:, 4:5])
                if debug:
                    nc.sync.dma_start(out=dbg["dbg_ab"][:, :], in_=sc2)

                # global top-16 of qq (squared space; monotone in importance)
                qqc = per.tile([128, NT], f32, tag="qqc")
                nc.vector.tensor_copy(out=qqc, in_=qq)
                pI = ptr.tile([NT, 128], f32, tag="tr")
                nc.tensor.transpose(pI, qqc, ident32)
                impt = per.tile([NT, 128], f32, tag="impt")
                nc.vector.tensor_copy(out=impt, in_=pI)
                it2 = top16_stage(impt, "itop_t2", NT)          # [16, 16]
                iflat = flatten_16x16(it2, "itop")              # [1, 256]
                qq16 = top16_stage(iflat, "itop_t3", 1)         # [1, 16] desc
                if debug:
                    nc.sync.dma_start(out=dbg["dbg_qq16"][:, :], in_=qq16)
                    nc.sync.dma_start(out=dbg["dbg_w16"][:, :], in_=w16v)

                # crossing in squared space: imp_(r) > w_(r)
                #   <=> qq_(r) > wadj_r = max((w_r - b)/a, 0)^2
                wadj = per.tile([1, B], f32, tag="wadj")
                nc.vector.tensor_scalar(out=wadj, in0=w16v, scalar1=sc2[:, 5:6],
                                        scalar2=None, op0=ALU.subtract)
                nc.vector.tensor_scalar(out=wadj, in0=wadj, scalar1=sc2[:, 6:7],
                                        scalar2=0.0, op0=ALU.mult, op1=ALU.max)
                nc.vector.tensor_mul(wadj, wadj, wadj)
                cross = per.tile([1, B], f32, tag="cross")
                nc.vector.tensor_tensor(out=cross, in0=qq16, in1=wadj, op=ALU.is_gt)
                rep = per.tile([1, B], f32, tag="rep")
                nc.vector.tensor_tensor_scan(out=rep, data0=cross, data1=cross,
                                             initial=1.0, op0=ALU.mult, op1=ALU.min)
                # thw0 = max selected w (raw);  thw1 = min selected qq (squared)
                selw = per.tile([1, B], f32, tag="selw")
                nc.vector.tensor_scalar(out=selw, in0=rep, scalar1=BIG, scalar2=-BIG,
                                        op0=ALU.mult, op1=ALU.add)
                nc.vector.tensor_mul(w16v, w16v, rep)
                nc.vector.tensor_add(selw, selw, w16v)
                thw = per.tile([1, 2], f32, tag="thw")
                nc.vector.tensor_reduce(out=thw[:, 0:1], in_=selw, axis=AX.X, op=ALU.max)
                seli = per.tile([1, B], f32, tag="seli")
                nc.vector.tensor_scalar(out=seli, in0=rep, scalar1=-BIG, scalar2=BIG,
                                        op0=ALU.mult, op1=ALU.add)
                nc.vector.tensor_mul(qq16, qq16, rep)
                nc.vector.tensor_add(seli, seli, qq16)
                nc.vector.tensor_reduce(out=thw[:, 1:2], in_=seli, axis=AX.X, op=ALU.min)
                if debug:
                    nc.sync.dma_start(out=dbg["dbg_thw"][:, :], in_=thw)
                thcol = per.tile([128, 2], f32, tag="thcol")
                nc.gpsimd.partition_broadcast(thcol, thw)

                # keep mask for local memory slots
                keep16 = per.tile([128, JT], bf16, tag="keep16")
                nc.vector.tensor_tensor(out=keep16, in0=wloc,
                                        in1=thcol[:, 0:1].to_broadcast([128, JT]),
                                        op=ALU.is_gt)
                if debug:
                    dk = per.tile([128, JT], f32, tag="dk")
                    nc.vector.tensor_copy(out=dk, in_=keep16)
                    nc.sync.dma_start(out=dbg["dbg_keep"][:, :], in_=dk)

                # local squared distances (bit-identical recompute from zk)
                ckT = [per.tile([128, QL], f16, tag=f"ckT_{c}", name=f"ckT_{c}") for c in range(2)]
                for c in range(2):
                    nc.vector.tensor_tensor(out=ckT[c], in0=zkT[c],
                                            in1=rmcol16[c].to_broadcast([128, QL]),
                                            op=ALU.subtract)
                XckT = [per.tile([128, QL], f16, tag=f"XckT_{c}", name=f"XckT_{c}") for c in range(2)]
                for dm in range(2):
                    pX = pre.tile([128, QL], f32, tag="acc")
                    for dc in range(2):
                        nc.tensor.matmul(pX, X[dc][:, dm * 128:(dm + 1) * 128],
                                         ckT[dc], start=(dc == 0), stop=(dc == 1))
                    nc.vector.tensor_copy(out=XckT[dm], in_=pX)
                Yk = [per.tile([128, QL], f16, tag=f"Yk_{c}", name=f"Yk_{c}") for c in range(2)]
                for c in range(2):
                    nc.vector.tensor_tensor(out=Yk[c], in0=ckT[c], in1=XckT[c],
                                            op=ALU.mult)
                qql_ps = qqp.tile([128, KT_Z], f32, tag="qqps")
                for t in range(KT_Z):
                    for dc in range(2):
                        nc.tensor.matmul(qql_ps[:, t:t + 1],
                                         Yk[dc][:, t * 128:(t + 1) * 128], onecol16,
                                         start=(dc == 0), stop=(dc == 1))
                qql = per.tile([128, KT_Z], f32, tag="qql")
                nc.vector.tensor_copy(out=qql, in_=qql_ps)
                nc.vector.tensor_scalar(out=qql, in0=qql, scalar1=1e-8, scalar2=None,
                                        op0=ALU.max)
                ins16 = per.tile([128, KT_Z], bf16, tag="ins16")
                nc.vector.tensor_tensor(out=ins16, in0=qql,
                                        in1=thcol[:, 1:2].to_broadcast([128, KT_Z]),
                                        op=ALU.is_ge)
                if debug:
                    di = per.tile([128, KT_Z], f32, tag="di")
                    nc.vector.tensor_copy(out=di, in_=ins16)
                    nc.sync.dma_start(out=dbg["dbg_ins"][:, :], in_=di)

                # exp bias columns
                biasall = per.tile([128, NJT], f32, tag="biasall")
                nc.vector.tensor_scalar(out=biasall[:, 0:JT], in0=keep16,
                                        scalar1=BIGM, scalar2=-(BIGM + SHIFT),
                                        op0=ALU.mult, op1=ALU.add)
                nc.vector.tensor_scalar(out=biasall[:, JT:NJT], in0=ins16,
                                        scalar1=BIGM, scalar2=-(BIGM + SHIFT),
                                        op0=ALU.mult, op1=ALU.add)

            # ---------------- flash attention (memory-sharded) ----------------
            qqp_ctx.__exit__(None, None, None)
            prj_ctx.__exit__(None, None, None)
            ptr_ctx.__exit__(None, None, None)
            pre_ctx.__exit__(None, None, None)

            rs_in = [dram.tile([1024, D + 1], bf16, tag=f"rs_in_{h}",
                               name=f"rs_in_{h}") for h in range(2)]
            rs_out = [dram.tile([128, D + 1], bf16, tag=f"rs_out_{h}",
                                name=f"rs_out_{h}") for h in range(2)]

            with (
                tc.tile_pool(name="att_sc", bufs=2, space="PSUM") as aps,
                tc.tile_pool(name="att_num", bufs=2, space="PSUM") as nps,
                tc.tile_pool(name="att_fin", bufs=2, space="PSUM") as fps,
            ):
                def emit_loop(qc):
                    num_ps = [nps.tile([128, 512], f32, tag=f"num{d}",
                                       name=f"num{d}_{qc}") for d in range(2)]
                    den_v = wrk.tile([128, 512], f32, tag="den_v",
                                     name=f"den_v_{qc}")
                    for jt in range(NJT):
                        if jt < JT:
                            kT_src, voff = memT, jt * 128
                            vlhs = mem16b[jt]
                        else:
                            kT_src, voff = zkT, (jt - JT) * 128
                            vlhs = zk16b[jt - JT]
                        sc_ps = aps.tile([128, 512], f32, tag="sc")
                        for dc in range(2):
                            nc.tensor.matmul(sc_ps,
                                             kT_src[dc][:, voff:voff + 128],
                                             QWT[dc][:, qc * 512:(qc + 1) * 512],
                                             start=(dc == 0), stop=(dc == 1))
                        e = wrk.tile([128, 512], bf16, tag="e")
                        nc.scalar.activation(out=e, in_=sc_ps, func=AF.Exp,
                                             bias=biasall[:, jt:jt + 1])
                        first, last = (jt == 0), (jt == NJT - 1)
                        for dc2 in range(2):
                            nc.tensor.matmul(num_ps[dc2],
                                             vlhs[:, dc2 * 128:(dc2 + 1) * 128], e,
                                             start=first, stop=last)
                        if first:
                            nc.vector.tensor_copy(out=den_v, in_=e)
                        else:
                            nc.vector.tensor_tensor(out=den_v, in0=den_v, in1=e,
                                                    op=ALU.add)
                    return num_ps, den_v

                def emit_post(qc, num_ps, den_v):
                    # numW psum -> sbuf (bf16) for the Wv application
                    numW = []
                    for dc in range(2):
                        t = wrk.tile([128, 512], bf16, tag=f"numW{dc}",
                                     name=f"numW{dc}_{qc}")
                        nc.vector.tensor_copy(out=t, in_=num_ps[dc])
                        numW.append(t)
                    # per-128q finalize partials: [128, 257] = Wv^T numW | 2*den
                    half, part = qc // 2, qc % 2
                    for qq_ in range(4):
                        fin = fps.tile([128, D + 1], f32, tag="fin",
                                       name=f"fin_{qc}_{qq_}")
                        for dc in range(2):
                            nc.tensor.matmul(fin[:, 0:D],
                                             numW[dc][:, qq_ * 128:(qq_ + 1) * 128],
                                             wvT[dc], start=(dc == 0), stop=(dc == 1))
                        nc.tensor.matmul(fin[:, D:D + 1],
                                         den_v[:, qq_ * 128:(qq_ + 1) * 128],
                                         onecol32, start=True, stop=True)
                        cp = wrk.tile([128, D + 1], bf16, tag="fincp",
                                      name=f"fincp_{qc}_{qq_}")
                        nc.scalar.copy(out=cp[:, 0:D], in_=fin[:, 0:D])
                        nc.scalar.mul(out=cp[:, D:D + 1], in_=fin[:, D:D + 1], mul=2.0)
                        base = part * 512 + qq_ * 128
                        nc.sync.dma_start(out=rs_in[half][base:base + 64, :],
                                          in_=cp[0:64, :])
                        nc.sync.dma_start(out=rs_in[half][base + 64:base + 128, :],
                                          in_=cp[64:128, :])
                    if part == 1:
                        nc.gpsimd.collective_compute(
                            "ReduceScatter", ALU.add,
                            replica_groups=[list(range(NC))],
                            ins=[rs_in[half][:, :].opt()],
                            outs=[rs_out[half][:, :].opt()],
                        )

                # software pipeline: chunk qc's post-processing is emitted
                # after chunk qc+1's flash loop so its drain overlaps compute
                state = {}
                for qc in range(4):
                    state[qc] = emit_loop(qc)
                    if qc >= 1:
                        with tc.high_priority():
                            emit_post(qc - 1, *state[qc - 1])
                with tc.high_priority():
                    emit_post(3, *state[3])

                # ---------------- finalize: two 128-row output halves --------
                # Pinned late in the simulated timeline so these (collective-
                # gated) ops land after all attention work in the engine queues.
                for h in range(2):
                    with tc.tile_wait_until(0.5 + 0.01 * h):
                        fo = per.tile([128, D + 1], bf16, tag=f"fo_{h}")
                        nc.sync.dma_start(out=fo[0:64, :], in_=rs_out[h][0:64, :])
                        nc.sync.dma_start(out=fo[64:128, :], in_=rs_out[h][64:128, :])
                        rec = per.tile([128, 1], f32, tag=f"rec_{h}")
                        nc.vector.reciprocal(out=rec, in_=fo[:, D:D + 1])
                        osb = per.tile([128, D], f32, tag=f"osb_{h}")
                        nc.vector.scalar_tensor_tensor(
                            out=osb, in0=fo[:, 0:D], scalar=rec, in1=zkadj[h],
                            op0=ALU.mult, op1=ALU.add)
                        nc.sync.dma_start(out=out_ext[h * 128:h * 128 + 64, :],
                                          in_=osb[0:64, :])
                        nc.sync.dma_start(out=out_ext[h * 128 + 64:(h + 1) * 128, :],
                                          in_=osb[64:128, :])

    nc.compile()
    return nc


_NC_CACHE: list = []


def _get_nc() -> bacc.Bacc:
    if not _NC_CACHE:
        _NC_CACHE.append(build())
    return _NC_CACHE[0]


def _pack_tiles(a: np.ndarray) -> np.ndarray:
    # [T*128, C] -> [128, T*C] partition-major pack
    t = a.shape[0] // 128
    return np.ascontiguousarray(
        a.reshape(t, 128, a.shape[1]).transpose(1, 0, 2).reshape(128, -1))


def _make_in_maps(inputs: dict) -> list[dict[str, np.ndarray]]:
    z = np.asarray(inputs["z"], dtype=np.float32)
    labels = np.asarray(inputs["labels"]).astype(np.int32).reshape(1, N)
    memory = np.asarray(inputs["memory"], dtype=np.float32)
    mw = np.asarray(inputs["memory_weights"], dtype=np.float32).reshape(-1)
    rmean = np.asarray(inputs["running_mean"], dtype=np.float32).reshape(1, D)
    rcov = np.asarray(inputs["running_cov"], dtype=np.float32)
    bq = np.asarray(inputs["bq"], dtype=np.float32).reshape(-1)
    bv = np.asarray(inputs["bv"], dtype=np.float32).reshape(1, D)
    ws = {nm: np.asarray(inputs[nm], dtype=np.float32) for nm in ("Wq", "Wk", "Wv")}

    # z pack with ones column per tile: [128, 16*257]
    zp = np.ones((16, 128, D + 1), np.float16)
    zp[:, :, 0:D] = z.reshape(16, 128, D).astype(np.float16)
    zp = np.ascontiguousarray(zp.transpose(1, 0, 2).reshape(128, ZW))

    wqp = _pack_tiles(ws["Wq"]).astype(np.float16)
    wkp = _pack_tiles(ws["Wk"]).astype(np.float16)
    # Wv^T packed, bf16: wvtp[p, c*D+j] = Wv[j, c*128+p]
    wvt = np.ascontiguousarray(ws["Wv"].T)
    wvtp = _pack_tiles(wvt).astype(ml_dtypes.bfloat16)
    # z^T packed: ztp[p, c*N+n] = z[n, c*128+p]
    ztp = _pack_tiles(np.ascontiguousarray(z.T)).astype(np.float16)

    in_maps = []
    for c in range(NC):
        aux = np.empty((128, AUXW), np.float32)
        aux[:, AUX_MW:AUX_MW + 128] = mw.reshape(128, 128)
        aux[:, AUX_WLOC:AUX_WLOC + JT] = mw[c * JL:(c + 1) * JL].reshape(JT, 128).T
        aux[:, AUX_RCOV:AUX_RCOV + 512] = _pack_tiles(rcov)
        aux[:, AUX_BQ] = bq[0:128]
        aux[:, AUX_BQ + 1] = bq[128:256]
        aux[:, AUX_RM] = rmean[0, 0:128]
        aux[:, AUX_RM + 1] = rmean[0, 128:256]
        zk = np.concatenate([z[c * 128:(c + 1) * 128],
                             z[1024 + c * 128:1024 + (c + 1) * 128]], axis=0)
        zkp = _pack_tiles(zk)
        mloc = memory[c * JL:(c + 1) * JL]
        in_maps.append({
            "zp": zp,
            "ztp": ztp,
            "zktp": _pack_tiles(np.ascontiguousarray(zk.T)).astype(np.float16),
            "zkbp": zkp.astype(ml_dtypes.bfloat16),
            "zkf": zkp,
            "mtp": _pack_tiles(np.ascontiguousarray(mloc.T)).astype(np.float16),
            "mbp": _pack_tiles(mloc).astype(ml_dtypes.bfloat16),
            "wqp": wqp, "wkp": wkp, "wvtp": wvtp,
            "aux": np.ascontiguousarray(aux),
            "labels": labels,
            "rmrow": rmean,
            "bvrow": bv,
        })
    return in_maps


def run(inputs: dict, trace: bool = False):
    nc = _get_nc()
    in_maps = _make_in_maps(inputs)
    res = run_bass_kernel_spmd(nc, in_maps, core_ids=list(range(NC)), trace=trace)
    out = np.empty((N, D), np.float32)
    for c in range(NC):
        oc = res.results[c]["out"]
        out[c * 128:(c + 1) * 128] = oc[0:128]
        out[1024 + c * 128:1024 + (c + 1) * 128] = oc[128:256]
    return out, res


def kernel(**inputs) -> np.ndarray:
    out, _ = run(inputs)
    return out


# revision 25
# speedup vs baseline: 1.0523x; 1.0523x over previous
"""AnomalyAwareMemory Trainium2 kernel (8 NeuronCores, single SPMD NEFF).

v3 strategy:

* Projection folding: bk cancels in softmax, so scores = SC*(z@(Wq^T Wk))@mem^T
  -- no K projection.  num = Wv^T(mem^T e) + bv*den, so no V projection either.
* Host-packed partition-major inputs ([128, big] arrays, one large DMA
  descriptor per partition) -- kills the ~35us descriptor-bound load phase.
* X = 2I - A instead of Newton-Schulz: A = 0.99 I + 0.01 cov + eps is within
  ~8e-3 of I, so inv(A) = 2I - A + O(|E|^2 ~ 6e-5), below the fp16 noise the
  baseline's 3 NS iterations bottom out at.
* No [128,*] sqrt: top-16 / crossing run on squared distances (monotone);
  the weight list is transformed into squared space instead.  mu comes free
  from a ones-column on the z^T z stats matmul.  Mahalanobis row-dots via
  scalar_tensor_tensor with accum_out (one op per tile).
* Eviction as in the baseline: crossing count R of sorted importance vs
  sorted weights, value thresholds, exp-bias masking (evicted slots /
  non-inserted pseudo-keys get exp bias -(1e4+20) -> exact 0 in bf16).
* Memory-sharded flash attention, 512 queries per chunk, q-major [128,257]
  partials (Wv applied per chunk with numW-chunks as lhsT), den accumulated
  on vector+gpsimd.  Two ReduceScatters ([1024,257] f32): RS-A after chunk 1
  overlaps chunks 2-3; only RS-B (~19us) is exposed.  Finalize = reciprocal
  + one fused (num*rec + z + 0.5 bv) op, emitted under tile_wait_until so
  collective-gated ops land after all attention work in the engine queues.
"""

import ml_dtypes
import numpy as np

import concourse.bass as bass
import concourse.mybir as mybir
from concourse import bacc
from concourse.tile import TileContext
from concourse.masks import make_identity
from concourse.bass_utils import run_bass_kernel_spmd

f32 = mybir.dt.float32
f16 = mybir.dt.float16
bf16 = mybir.dt.bfloat16
i32 = mybir.dt.int32
AF = mybir.ActivationFunctionType
ALU = mybir.AluOpType
AX = mybir.AxisListType

N = 2048          # batch
D = 256           # embedding dim
MEM = 16384       # memory slots
NC = 8            # cores
JL = MEM // NC    # 2048 memory slots per core
QL = N // NC      # 256 output rows per core (2 chunks of 128)
NT = N // 128     # 16 z tiles
JT = JL // 128    # 16 local memory tiles
KT_Z = QL // 128  # 2 local z pseudo-key tiles
NJT = JT + KT_Z   # 18 flash tiles
B = 16            # top-B merge width
SHIFT = 20.0      # global exp shift, cancels in num/den
SC = 1.0 / (16.0 * 0.1)   # 1/(sqrt(D)*TEMP)
MOM = 0.01
BIG = 1e30
BIGM = 1e4
ZW = NT * (D + 1)         # packed z columns (ones col per tile)
# aux pack layout (f32 [128, AUXW])
AUX_MW = 0                # [128, 128] memory weights
AUX_WLOC = 128            # [128, JT] local weights col-per-tile
AUX_RCOV = 144            # [128, 512] running_cov (2 chunks)
AUX_BQ = 656              # 2 cols: bq as columns
AUX_RM = 658              # 2 cols: running_mean as columns
AUXW = 660


def build(debug: bool = False) -> bacc.Bacc:
    nc = bacc.Bacc(num_devices=NC)

    zp_ext = nc.declare_dram_parameter("zp", [128, ZW], f16, isOutput=False)
    ztp_ext = nc.declare_dram_parameter("ztp", [128, 2 * N], f16, isOutput=False)
    mtp_ext = nc.declare_dram_parameter("mtp", [128, 2 * JL], f16, isOutput=False)
    mbp_ext = nc.declare_dram_parameter("mbp", [128, JT * D], bf16, isOutput=False)
    zktp_ext = nc.declare_dram_parameter("zktp", [128, 2 * QL], f16, isOutput=False)
    zkbp_ext = nc.declare_dram_parameter("zkbp", [128, KT_Z * D], bf16, isOutput=False)
    zkf_ext = nc.declare_dram_parameter("zkf", [128, KT_Z * D], f32, isOutput=False)
    wvtp_ext = nc.declare_dram_parameter("wvtp", [128, 2 * D], bf16, isOutput=False)
    wqp_ext = nc.declare_dram_parameter("wqp", [128, 512], f16, isOutput=False)
    wkp_ext = nc.declare_dram_parameter("wkp", [128, 512], f16, isOutput=False)
    aux_ext = nc.declare_dram_parameter("aux", [128, AUXW], f32, isOutput=False)
    lab_ext = nc.declare_dram_parameter("labels", [1, N], i32, isOutput=False)
    rmr_ext = nc.declare_dram_parameter("rmrow", [1, D], f32, isOutput=False)
    bvr_ext = nc.declare_dram_parameter("bvrow", [1, D], f32, isOutput=False)
    out_ext = nc.declare_dram_parameter("out", [QL, D], f32, isOutput=True)
    dbg = {}
    if debug:
        for nm, shp in [("dbg_S", [128, D]), ("dbg_X", [128, D]),
                        ("dbg_qq", [128, NT]), ("dbg_qq16", [1, B]),
                        ("dbg_w16", [1, B]), ("dbg_thw", [1, 2]),
                        ("dbg_keep", [128, JT]), ("dbg_ins", [128, KT_Z]),
                        ("dbg_QWT", [128, 512]), ("dbg_mu", [1, D]),
                        ("dbg_ab", [1, 8]), ("dbg_W2", [128, D])]:
            dbg[nm] = nc.declare_dram_parameter(nm, shp, f32, isOutput=True)

    with TileContext(nc) as tc:
        with (
            tc.tile_pool(name="per", bufs=1) as per,          # persistent sbuf
            tc.tile_pool(name="wrk", bufs=4) as wrk,          # rotating sbuf
            tc.tile_pool(name="dram", bufs=1, space="DRAM") as dram,
        ):
            # phase-scoped PSUM pools (closed before attention pools open)
            pre_ctx = tc.tile_pool(name="pre_ps", bufs=3, space="PSUM")
            pre = pre_ctx.__enter__()
            ptr_ctx = tc.tile_pool(name="ptr", bufs=2, space="PSUM")
            ptr = ptr_ctx.__enter__()
            prj_ctx = tc.tile_pool(name="prj_ps", bufs=2, space="PSUM")
            prj = prj_ctx.__enter__()
            qqp_ctx = tc.tile_pool(name="qq_ps", bufs=1, space="PSUM")
            qqp = qqp_ctx.__enter__()

            # ---------------- input DMAs (z first: stats gate phase A) ------
            zbig = per.tile([128, ZW], f16, tag="zbig")
            for i in range(2):
                w = ZW // 2
                nc.sync.dma_start(out=zbig[:, i * w:(i + 1) * w],
                                  in_=zp_ext[:, i * w:(i + 1) * w])
            z16 = [zbig[:, t * (D + 1):(t + 1) * (D + 1)] for t in range(NT)]

            labi = per.tile([1, N], i32, tag="labi")
            nc.sync.dma_start(out=labi, in_=lab_ext[:, :])
            aux = per.tile([128, AUXW], f32, tag="aux")
            nc.sync.dma_start(out=aux, in_=aux_ext[:, :])
            rmrow = per.tile([1, D], f32, tag="rmrow")
            nc.sync.dma_start(out=rmrow, in_=rmr_ext[:, :])
            bvrow = per.tile([1, D], f32, tag="bvrow")
            nc.sync.dma_start(out=bvrow, in_=bvr_ext[:, :])

            ztb = per.tile([128, 2 * N], f16, tag="ztb")
            for i in range(2):
                w = 2 * N // 2
                nc.sync.dma_start(out=ztb[:, i * w:(i + 1) * w],
                                  in_=ztp_ext[:, i * w:(i + 1) * w])
            zT = [ztb[:, c * N:(c + 1) * N] for c in range(2)]

            wqb = per.tile([128, 512], f16, tag="wqb")
            nc.gpsimd.dma_start(out=wqb, in_=wqp_ext[:, :])
            wkb = per.tile([128, 512], f16, tag="wkb")
            nc.gpsimd.dma_start(out=wkb, in_=wkp_ext[:, :])
            wvtb = per.tile([128, 2 * D], bf16, tag="wvtb")
            nc.gpsimd.dma_start(out=wvtb, in_=wvtp_ext[:, :])
            wvT = [wvtb[:, c * D:(c + 1) * D] for c in range(2)]
            wq16 = [wqb[:, c * D:(c + 1) * D] for c in range(2)]
            wk16 = [wkb[:, c * D:(c + 1) * D] for c in range(2)]

            zktb = per.tile([128, 2 * QL], f16, tag="zktb")
            nc.gpsimd.dma_start(out=zktb, in_=zktp_ext[:, :])
            zkT = [zktb[:, c * QL:(c + 1) * QL] for c in range(2)]
            zkbb = per.tile([128, KT_Z * D], bf16, tag="zkbb")
            nc.gpsimd.dma_start(out=zkbb, in_=zkbp_ext[:, :])
            zk16b = [zkbb[:, t * D:(t + 1) * D] for t in range(KT_Z)]
            zk32 = per.tile([128, KT_Z * D], f32, tag="zk32")
            nc.scalar.dma_start(out=zk32, in_=zkf_ext[:, :])

            mtb = per.tile([128, 2 * JL], f16, tag="mtb")
            for i in range(2):
                w = 2 * JL // 2
                nc.scalar.dma_start(out=mtb[:, i * w:(i + 1) * w],
                                    in_=mtp_ext[:, i * w:(i + 1) * w])
            memT = [mtb[:, c * JL:(c + 1) * JL] for c in range(2)]
            mbb = per.tile([128, JT * D], bf16, tag="mbb")
            for i in range(2):
                w = JT * D // 2
                nc.scalar.dma_start(out=mbb[:, i * w:(i + 1) * w],
                                    in_=mbp_ext[:, i * w:(i + 1) * w])
            mem16b = [mbb[:, t * D:(t + 1) * D] for t in range(JT)]

            wfull = aux[:, AUX_MW:AUX_MW + 128]
            wloc = aux[:, AUX_WLOC:AUX_WLOC + JT]
            rcov_s = []
            for c in range(2):
                t = per.tile([128, D], f32, tag=f"rcov_{c}")
                nc.scalar.mul(out=t, in_=aux[:, AUX_RCOV + c * D:AUX_RCOV + (c + 1) * D],
                              mul=1.0 - MOM)
                rcov_s.append(t)
            bqcol16 = []
            for c in range(2):
                t = per.tile([128, 1], f16, tag=f"bqcol16_{c}")
                nc.scalar.copy(out=t, in_=aux[:, AUX_BQ + c:AUX_BQ + c + 1])
                bqcol16.append(t)

            # ---------------- constants ----------------
            ident32 = per.tile([128, 128], f32, tag="ident32")
            make_identity(nc, ident32)
            ones11 = per.tile([1, 1], f32, tag="ones11")
            nc.vector.memset(ones11, 1.0)
            onecol32 = per.tile([128, 1], f32, tag="onecol32")
            nc.vector.memset(onecol32, 1.0)
            I2 = []     # 2*I (f16) row chunk c
            for c in range(2):
                t2 = per.tile([128, D], f16, tag=f"I2_{c}")
                nc.gpsimd.memset(t2, 0.0)
                nc.gpsimd.affine_select(out=t2, in_=t2, compare_op=ALU.not_equal,
                                        fill=2.0, base=128 * c,
                                        pattern=[[-1, D]], channel_multiplier=1)
                I2.append(t2)

            # residual rows + 0.5*bv, one [128, D] tile per output half
            bvrep = per.tile([128, D], f32, tag="bvrep")
            nc.gpsimd.partition_broadcast(bvrep, bvrow)
            halfbv = per.tile([128, D], f32, tag="halfbv")
            nc.scalar.mul(out=halfbv, in_=bvrep, mul=0.5)
            zkadj = []
            for h in range(KT_Z):
                t = per.tile([128, D], f32, tag=f"zkadj_{h}")
                nc.vector.tensor_tensor(out=t, in0=zk32[:, h * D:(h + 1) * D],
                                         in1=halfbv, op=ALU.add)
                zkadj.append(t)

            # ---------------- W2 = Wq^T @ Wk;  bqwk = bq @ Wk ----------------
            W2 = []
            for dm in range(2):
                ps = prj.tile([128, D], f32, tag="acc")
                for kc in range(2):
                    nc.tensor.matmul(ps, wq16[kc][:, dm * 128:(dm + 1) * 128],
                                     wk16[kc], start=(kc == 0), stop=(kc == 1))
                t = per.tile([128, D], f16, tag=f"W2_{dm}")
                nc.scalar.copy(out=t, in_=ps)
                W2.append(t)
            if debug:
                dw2 = per.tile([128, D], f32, tag="dw2")
                nc.vector.tensor_copy(out=dw2, in_=W2[0])
                nc.sync.dma_start(out=dbg["dbg_W2"][:, :], in_=dw2)
            ps_bq = pre.tile([1, D], f32, tag="acc")
            for kc in range(2):
                nc.tensor.matmul(ps_bq, bqcol16[kc], wk16[kc],
                                 start=(kc == 0), stop=(kc == 1))
            bqwk_row = per.tile([1, D], f32, tag="bqwk_row")
            nc.vector.tensor_scalar(out=bqwk_row, in0=ps_bq, scalar1=SC,
                                    scalar2=None, op0=ALU.mult)
            bqwk_col = []
            for c in range(2):
                p = ptr.tile([128, 1], f32, tag="tr")
                nc.tensor.matmul(p, bqwk_row[0:1, c * 128:(c + 1) * 128], ones11,
                                 start=True, stop=True)
                t = per.tile([128, 1], f32, tag=f"bqwk_col_{c}")
                nc.vector.tensor_copy(out=t, in_=p)
                bqwk_col.append(t)

            # ---------------- QWT = SC * (W2^T z^T + bqwk^T) ----------------
            QWT = [per.tile([128, N], f16, tag=f"QWT_{c}", name=f"QWT_{c}") for c in range(2)]
            for dm in range(2):
                for qc in range(N // 512):
                    ps = prj.tile([128, 512], f32, tag="acc")
                    for dc in range(2):
                        nc.tensor.matmul(ps, W2[dc][:, dm * 128:(dm + 1) * 128],
                                         zT[dc][:, qc * 512:(qc + 1) * 512],
                                         start=(dc == 0), stop=(dc == 1))
                    nc.scalar.activation(out=QWT[dm][:, qc * 512:(qc + 1) * 512],
                                         in_=ps, func=AF.Identity,
                                         bias=bqwk_col[dm], scale=SC)
            if debug:
                dq = per.tile([128, 512], f32, tag="dqw")
                nc.vector.tensor_copy(out=dq, in_=QWT[0][:, 0:512])
                nc.sync.dma_start(out=dbg["dbg_QWT"][:, :], in_=dq)

            # ---------------- phase A: stats -> thresholds -> exp biases ----
            with tc.high_priority():
                # KL(label dist || uniform)
                sc2 = per.tile([1, 8], f32, tag="sc2")  # [dmin dmax rden kl a b 1/a _]
                labf = per.tile([1, N], f32, tag="labf")
                nc.vector.tensor_copy(out=labf, in_=labi)
                cnt1 = per.tile([1, 1], f32, tag="cnt1")
                nc.vector.tensor_reduce(out=cnt1, in_=labf, axis=AX.X, op=ALU.add)
                pvec = per.tile([1, 2], f32, tag="pvec")
                nc.vector.tensor_scalar(out=pvec[:, 1:2], in0=cnt1, scalar1=1.0 / N,
                                        scalar2=None, op0=ALU.mult)
                nc.vector.tensor_scalar(out=pvec[:, 0:1], in0=pvec[:, 1:2],
                                        scalar1=-1.0, scalar2=1.0,
                                        op0=ALU.mult, op1=ALU.add)
                lnin = per.tile([1, 2], f32, tag="lnin")
                nc.vector.tensor_scalar(out=lnin, in0=pvec, scalar1=2.0, scalar2=1e-8,
                                        op0=ALU.mult, op1=ALU.max)
                lnv = per.tile([1, 2], f32, tag="lnv")
                nc.scalar.activation(out=lnv, in_=lnin, func=AF.Ln)
                terms = per.tile([1, 2], f32, tag="terms")
                nc.vector.tensor_mul(terms, pvec, lnv)
                klr = per.tile([1, 1], f32, tag="klr")
                nc.vector.tensor_reduce(out=klr, in_=terms, axis=AX.X, op=ALU.add)
                nc.vector.tensor_scalar(out=sc2[:, 3:4], in0=klr, scalar1=0.0,
                                        scalar2=None, op0=ALU.max)

                def top16_stage(cur, tag, pdim):
                    tb = per.tile([pdim, B], f32, tag=tag)
                    for r in range(2):
                        nc.vector.max(out=tb[:, r * 8:(r + 1) * 8], in_=cur)
                        nc.vector.match_replace(out=cur, in_to_replace=tb[:, r * 8:(r + 1) * 8],
                                                in_values=cur, imm_value=-BIG)
                    return tb

                def flatten_16x16(tb, tag):
                    db = dram.tile([B, B], f32, tag=f"{tag}_d")
                    nc.sync.dma_start(out=db, in_=tb)
                    flat = per.tile([1, B * B], f32, tag=f"{tag}_f")
                    nc.sync.dma_start(
                        out=flat,
                        in_=db.rearrange("p f -> (p f)").rearrange(
                            "(a b) -> a b", a=1))
                    return flat

                def global_top16(src128, tag):
                    t1 = top16_stage(src128, f"{tag}_t1", 128)      # [128, 16]
                    pT = ptr.tile([B, 128], f32, tag="tr")
                    nc.tensor.transpose(pT, t1, ident32)
                    t1t = per.tile([B, 128], f32, tag=f"{tag}_tt")
                    nc.vector.tensor_copy(out=t1t, in_=pT)
                    t2 = top16_stage(t1t, f"{tag}_t2", B)           # [16, 16]
                    flat = flatten_16x16(t2, tag)                   # [1, 256]
                    return top16_stage(flat, f"{tag}_t3", 1)        # [1, 16]

                # weights bottom-16 (ascending): independent, runs off aux
                wneg = per.tile([128, 128], f32, tag="wneg")
                nc.vector.tensor_scalar(out=wneg, in0=wfull, scalar1=-1.0,
                                        scalar2=None, op0=ALU.mult)
                w16neg = global_top16(wneg, "wtop")
                w16v = per.tile([1, B], f32, tag="w16v")
                nc.vector.tensor_scalar(out=w16v, in0=w16neg, scalar1=-1.0,
                                        scalar2=None, op0=ALU.mult)

                # ---- mu first (gates rmcol/cT), then S = z^T z ----
                onecol16 = per.tile([128, 1], f16, tag="onecol16")
                nc.vector.memset(onecol16, 1.0)
                pmu = pre.tile([1, D + 1], f32, tag="acc")
                for t in range(NT):
                    nc.tensor.matmul(pmu, onecol16, z16[t],
                                     start=(t == 0), stop=(t == NT - 1))
                murow = per.tile([1, D], f32, tag="murow")
                nc.vector.tensor_scalar(out=murow, in0=pmu[0:1, 0:D],
                                        scalar1=1.0 / N, scalar2=None,
                                        op0=ALU.mult)
                mucol = []
                for c in range(2):
                    p = ptr.tile([128, 1], f32, tag="tr")
                    nc.tensor.matmul(p, murow[0:1, c * 128:(c + 1) * 128], ones11,
                                     start=True, stop=True)
                    t = per.tile([128, 1], f32, tag=f"mucol_{c}")
                    nc.vector.tensor_copy(out=t, in_=p)
                    mucol.append(t)
                S_sb = []
                for mc in range(2):
                    ps = pre.tile([128, D], f32, tag="acc")
                    for t in range(NT):
                        nc.tensor.matmul(ps, z16[t][:, mc * 128:(mc + 1) * 128],
                                         z16[t][:, 0:D],
                                         start=(t == 0), stop=(t == NT - 1))
                    sb = per.tile([128, D], f32, tag=f"S_{mc}")
                    nc.vector.tensor_scalar(out=sb, in0=ps,
                                            scalar1=MOM / (N - 1),
                                            scalar2=None, op0=ALU.mult)
                    S_sb.append(sb)
                if debug:
                    ds = per.tile([128, D], f32, tag="ds")
                    nc.vector.tensor_copy(out=ds, in_=S_sb[0])
                    nc.sync.dma_start(out=dbg["dbg_S"][:, :], in_=ds)

                mu16 = per.tile([1, D], f16, tag="mu16")
                nc.scalar.copy(out=mu16, in_=murow)
                if debug:
                    nc.sync.dma_start(out=dbg["dbg_mu"][:, :], in_=murow)

                # rm row / cols / broadcast
                rm = per.tile([1, D], f32, tag="rm")
                nc.vector.tensor_scalar(out=rm, in0=rmrow, scalar1=1.0 - MOM,
                                        scalar2=None, op0=ALU.mult)
                musc = per.tile([1, D], f32, tag="musc")
                nc.vector.tensor_scalar(out=musc, in0=murow, scalar1=MOM,
                                        scalar2=None, op0=ALU.mult)
                nc.vector.tensor_add(rm, rm, musc)
                rmcol = []
                for c in range(2):
                    t = per.tile([128, 1], f32, tag=f"rmcol_{c}")
                    nc.vector.tensor_scalar(
                        out=t, in0=aux[:, AUX_RM + c:AUX_RM + c + 1],
                        scalar1=1.0 - MOM, scalar2=None, op0=ALU.mult)
                    t2 = per.tile([128, 1], f32, tag=f"rmcol2_{c}")
                    nc.vector.tensor_scalar(out=t2, in0=mucol[c], scalar1=MOM,
                                            scalar2=None, op0=ALU.mult)
                    nc.vector.tensor_add(t, t, t2)
                    rmcol.append(t)

                # ---- X = 2I - A,  A = (1-mom)*rcov + mom*cov ----
                X = []
                for mc in range(2):
                    pmo = pre.tile([128, D], f32, tag="acc")
                    nc.tensor.matmul(pmo, mu16[:, mc * 128:(mc + 1) * 128], mu16,
                                     start=True, stop=True)
                    acc = per.tile([128, D], f32, tag=f"A32_{mc}")
                    nc.vector.tensor_add(acc, S_sb[mc], rcov_s[mc])
                    nc.vector.scalar_tensor_tensor(
                        out=acc, in0=pmo, scalar=-MOM * N / (N - 1), in1=acc,
                        op0=ALU.mult, op1=ALU.add)
                    x = per.tile([128, D], f16, tag=f"X_{mc}")
                    nc.vector.tensor_tensor(out=x, in0=I2[mc], in1=acc,
                                            op=ALU.subtract)
                    X.append(x)
                if debug:
                    dx = per.tile([128, D], f32, tag="dx")
                    nc.vector.tensor_copy(out=dx, in_=X[0])
                    nc.sync.dma_start(out=dbg["dbg_X"][:, :], in_=dx)

                # ---- Mahalanobis squared distances (all N) ----
                rmcol16 = []
                for c in range(2):
                    t = per.tile([128, 1], f16, tag=f"rmcol16_{c}")
                    nc.vector.tensor_copy(out=t, in_=rmcol[c])
                    rmcol16.append(t)
                cT = [per.tile([128, N], f16, tag=f"cT_{c}", name=f"cT_{c}") for c in range(2)]
                for c in range(2):
                    for hh in range(2):
                        nc.vector.tensor_tensor(
                            out=cT[c][:, hh * 1024:(hh + 1) * 1024],
                            in0=zT[c][:, hh * 1024:(hh + 1) * 1024],
                            in1=rmcol16[c].to_broadcast([128, 1024]),
                            op=ALU.subtract)
                # X symmetric: qq[n] = sum_d cT[d,n] * (X cT)[d,n], summed on PE
                XcT = [per.tile([128, N], f16, tag=f"XcT_{c}", name=f"XcT_{c}")
                       for c in range(2)]
                for dm in range(2):
                    for ns in range(4):
                        pX = pre.tile([128, 512], f32, tag="acc")
                        for dc in range(2):
                            nc.tensor.matmul(pX, X[dc][:, dm * 128:(dm + 1) * 128],
                                             cT[dc][:, ns * 512:(ns + 1) * 512],
                                             start=(dc == 0), stop=(dc == 1))
                        dst = XcT[dm][:, ns * 512:(ns + 1) * 512]
                        if ns % 2 == 0:
                            nc.scalar.copy(out=dst, in_=pX)
                        else:
                            nc.vector.tensor_copy(out=dst, in_=pX)
                Y = [per.tile([128, N], f16, tag=f"Y_{c}", name=f"Y_{c}")
                     for c in range(2)]
                for c in range(2):
                    for hh in range(2):
                        nc.vector.tensor_tensor(
                            out=Y[c][:, hh * 1024:(hh + 1) * 1024],
                            in0=cT[c][:, hh * 1024:(hh + 1) * 1024],
                            in1=XcT[c][:, hh * 1024:(hh + 1) * 1024],
                            op=ALU.mult)
                qq_ps = qqp.tile([128, NT], f32, tag="qqps")
                for t in range(NT):
                    for dc in range(2):
                        nc.tensor.matmul(qq_ps[:, t:t + 1],
                                         Y[dc][:, t * 128:(t + 1) * 128], onecol16,
                                         start=(dc == 0), stop=(dc == 1))
                qq = per.tile([128, NT], f32, tag="qq")
                nc.vector.tensor_copy(out=qq, in_=qq_ps)
                nc.vector.tensor_scalar(out=qq, in0=qq, scalar1=1e-8, scalar2=None,
                                        op0=ALU.max)
                if debug:
                    nc.sync.dma_start(out=dbg["dbg_qq"][:, :], in_=qq)

                # local squared distances (bit-identical recompute from zk)
                ckT = [per.tile([128, QL], f16, tag=f"ckT_{c}", name=f"ckT_{c}") for c in range(2)]
                for c in range(2):
                    nc.vector.tensor_tensor(out=ckT[c], in0=zkT[c],
                                            in1=rmcol16[c].to_broadcast([128, QL]),
                                            op=ALU.subtract)
                XckT = [per.tile([128, QL], f16, tag=f"XckT_{c}", name=f"XckT_{c}") for c in range(2)]
                for dm in range(2):
                    pX = pre.tile([128, QL], f32, tag="acc")
                    for dc in range(2):
                        nc.tensor.matmul(pX, X[dc][:, dm * 128:(dm + 1) * 128],
                                         ckT[dc], start=(dc == 0), stop=(dc == 1))
                    nc.vector.tensor_copy(out=XckT[dm], in_=pX)
                Yk = [per.tile([128, QL], f16, tag=f"Yk_{c}", name=f"Yk_{c}") for c in range(2)]
                for c in range(2):
                    nc.vector.tensor_tensor(out=Yk[c], in0=ckT[c], in1=XckT[c],
                                            op=ALU.mult)
                qql_ps = qqp.tile([128, KT_Z], f32, tag="qqps")
                for t in range(KT_Z):
                    for dc in range(2):
                        nc.tensor.matmul(qql_ps[:, t:t + 1],
                                         Yk[dc][:, t * 128:(t + 1) * 128], onecol16,
                                         start=(dc == 0), stop=(dc == 1))
                qql = per.tile([128, KT_Z], f32, tag="qql")
                nc.vector.tensor_copy(out=qql, in_=qql_ps)
                nc.vector.tensor_scalar(out=qql, in0=qql, scalar1=1e-8, scalar2=None,
                                        op0=ALU.max)
                # dmin/dmax from squared extremes (single tiny sqrt)
                dmm = per.tile([128, 2], f32, tag="dmm")
                nc.vector.tensor_reduce(out=dmm[:, 0:1], in_=qq, axis=AX.X, op=ALU.min)
                nc.vector.tensor_reduce(out=dmm[:, 1:2], in_=qq, axis=AX.X, op=ALU.max)
                qex = per.tile([1, 2], f32, tag="qex")
                for k, op in ((0, ALU.min), (1, ALU.max)):
                    p = ptr.tile([1, 128], f32, tag="tr")
                    nc.tensor.transpose(p, dmm[:, k:k + 1], ident32)
                    row = per.tile([1, 128], f32, tag=f"drow_{k}")
                    nc.vector.tensor_copy(out=row, in_=p)
                    nc.vector.tensor_reduce(out=qex[:, k:k + 1], in_=row, axis=AX.X, op=op)
                nc.scalar.activation(out=sc2[:, 0:2], in_=qex, func=AF.Sqrt)
                # exp table warm-up, tied to the sqrt result so it runs here
                warm = per.tile([1, 1], f32, tag="warm")
                nc.scalar.activation(out=warm, in_=sc2[:, 0:1], func=AF.Exp,
                                     scale=0.0)

                # rden = 1/(dmax-dmin+1e-8); a = rden*kl; b = (1-dmin*rden)*kl
                dd = per.tile([1, 1], f32, tag="dd")
                nc.vector.tensor_sub(dd, sc2[:, 1:2], sc2[:, 0:1])
                nc.vector.tensor_scalar(out=dd, in0=dd, scalar1=1e-8, scalar2=None,
                                        op0=ALU.add)
                nc.vector.reciprocal(out=sc2[:, 2:3], in_=dd)
                nc.vector.tensor_mul(sc2[:, 4:5], sc2[:, 2:3], sc2[:, 3:4])
                t5 = per.tile([1, 1], f32, tag="t5")
                nc.vector.tensor_mul(t5, sc2[:, 0:1], sc2[:, 2:3])
                nc.vector.tensor_scalar(out=t5, in0=t5, scalar1=-1.0, scalar2=1.0,
                                        op0=ALU.mult, op1=ALU.add)
                nc.vector.tensor_mul(sc2[:, 5:6], t5, sc2[:, 3:4])
                nc.vector.reciprocal(out=sc2[:, 6:7], in_=sc2[:, 4:5])
                if debug:
                    nc.sync.dma_start(out=dbg["dbg_ab"][:, :], in_=sc2)

                # global top-16 of qq (squared space; monotone in importance)
                qqc = per.tile([128, NT], f32, tag="qqc")
                nc.vector.tensor_copy(out=qqc, in_=qq)
                pI = ptr.tile([NT, 128], f32, tag="tr")
                nc.tensor.transpose(pI, qqc, ident32)
                impt = per.tile([NT, 128], f32, tag="impt")
                nc.vector.tensor_copy(out=impt, in_=pI)
                it2 = top16_stage(impt, "itop_t2", NT)          # [16, 16]
                iflat = flatten_16x16(it2, "itop")              # [1, 256]
                qq16 = top16_stage(iflat, "itop_t3", 1)         # [1, 16] desc
                if debug:
                    nc.sync.dma_start(out=dbg["dbg_qq16"][:, :], in_=qq16)
                    nc.sync.dma_start(out=dbg["dbg_w16"][:, :], in_=w16v)

                # crossing in squared space: imp_(r) > w_(r)
                #   <=> qq_(r) > wadj_r = max((w_r - b)/a, 0)^2
                wadj = per.tile([1, B], f32, tag="wadj")
                nc.vector.tensor_scalar(out=wadj, in0=w16v, scalar1=sc2[:, 5:6],
                                        scalar2=None, op0=ALU.subtract)
                nc.vector.tensor_scalar(out=wadj, in0=wadj, scalar1=sc2[:, 6:7],
                                        scalar2=0.0, op0=ALU.mult, op1=ALU.max)
                nc.vector.tensor_mul(wadj, wadj, wadj)
                cross = per.tile([1, B], f32, tag="cross")
                nc.vector.tensor_tensor(out=cross, in0=qq16, in1=wadj, op=ALU.is_gt)
                rep = per.tile([1, B], f32, tag="rep")
                nc.vector.tensor_tensor_scan(out=rep, data0=cross, data1=cross,
                                             initial=1.0, op0=ALU.mult, op1=ALU.min)
                # thw0 = max selected w (raw);  thw1 = min selected qq (squared)
                selw = per.tile([1, B], f32, tag="selw")
                nc.vector.tensor_scalar(out=selw, in0=rep, scalar1=BIG, scalar2=-BIG,
                                        op0=ALU.mult, op1=ALU.add)
                nc.vector.tensor_mul(w16v, w16v, rep)
                nc.vector.tensor_add(selw, selw, w16v)
                thw = per.tile([1, 2], f32, tag="thw")
                nc.vector.tensor_reduce(out=thw[:, 0:1], in_=selw, axis=AX.X, op=ALU.max)
                seli = per.tile([1, B], f32, tag="seli")
                nc.vector.tensor_scalar(out=seli, in0=rep, scalar1=-BIG, scalar2=BIG,
                                        op0=ALU.mult, op1=ALU.add)
                nc.vector.tensor_mul(qq16, qq16, rep)
                nc.vector.tensor_add(seli, seli, qq16)
                nc.vector.tensor_reduce(out=thw[:, 1:2], in_=seli, axis=AX.X, op=ALU.min)
                if debug:
                    nc.sync.dma_start(out=dbg["dbg_thw"][:, :], in_=thw)
                thcol = per.tile([128, 2], f32, tag="thcol")
                nc.gpsimd.partition_broadcast(thcol, thw)

                # keep mask for local memory slots
                keep16 = per.tile([128, JT], bf16, tag="keep16")
                nc.vector.tensor_tensor(out=keep16, in0=wloc,
                                        in1=thcol[:, 0:1].to_broadcast([128, JT]),
                                        op=ALU.is_gt)
                if debug:
                    dk = per.tile([128, JT], f32, tag="dk")
                    nc.vector.tensor_copy(out=dk, in_=keep16)
                    nc.sync.dma_start(out=dbg["dbg_keep"][:, :], in_=dk)

                ins16 = per.tile([128, KT_Z], bf16, tag="ins16")
                nc.vector.tensor_tensor(out=ins16, in0=qql,
                                        in1=thcol[:, 1:2].to_broadcast([128, KT_Z]),
                                        op=ALU.is_ge)
                if debug:
                    di = per.tile([128, KT_Z], f32, tag="di")
                    nc.vector.tensor_copy(out=di, in_=ins16)
                    nc.sync.dma_start(out=dbg["dbg_ins"][:, :], in_=di)

                # exp bias columns
                biasall = per.tile([128, NJT], f32, tag="biasall")
                nc.vector.tensor_scalar(out=biasall[:, 0:JT], in0=keep16,
                                        scalar1=BIGM, scalar2=-(BIGM + SHIFT),
                                        op0=ALU.mult, op1=ALU.add)
                nc.vector.tensor_scalar(out=biasall[:, JT:NJT], in0=ins16,
                                        scalar1=BIGM, scalar2=-(BIGM + SHIFT),
                                        op0=ALU.mult, op1=ALU.add)

            # ---------------- flash attention (memory-sharded) ----------------
            qqp_ctx.__exit__(None, None, None)
            prj_ctx.__exit__(None, None, None)
            ptr_ctx.__exit__(None, None, None)
            pre_ctx.__exit__(None, None, None)

            rs_in = [dram.tile([1024, D + 1], bf16, tag=f"rs_in_{h}",
                               name=f"rs_in_{h}") for h in range(2)]
            rs_out = [dram.tile([128, D + 1], bf16, tag=f"rs_out_{h}",
                                name=f"rs_out_{h}") for h in range(2)]

            with (
                tc.tile_pool(name="att_sc", bufs=2, space="PSUM") as aps,
                tc.tile_pool(name="att_num", bufs=2, space="PSUM") as nps,
                tc.tile_pool(name="att_fin", bufs=2, space="PSUM") as fps,
            ):
                def emit_loop(qc):
                    num_ps = [nps.tile([128, 512], f32, tag=f"num{d}",
                                       name=f"num{d}_{qc}") for d in range(2)]
                    den_v = wrk.tile([128, 512], f32, tag="den_v",
                                     name=f"den_v_{qc}")
                    for jt in range(NJT):
                        if jt < JT:
                            kT_src, voff = memT, jt * 128
                            vlhs = mem16b[jt]
                        else:
                            kT_src, voff = zkT, (jt - JT) * 128
                            vlhs = zk16b[jt - JT]
                        sc_ps = aps.tile([128, 512], f32, tag="sc")
                        for dc in range(2):
                            nc.tensor.matmul(sc_ps,
                                             kT_src[dc][:, voff:voff + 128],
                                             QWT[dc][:, qc * 512:(qc + 1) * 512],
                                             start=(dc == 0), stop=(dc == 1))
                        e = wrk.tile([128, 512], bf16, tag="e")
                        nc.scalar.activation(out=e, in_=sc_ps, func=AF.Exp,
                                             bias=biasall[:, jt:jt + 1])
                        first, last = (jt == 0), (jt == NJT - 1)
                        for dc2 in range(2):
                            nc.tensor.matmul(num_ps[dc2],
                                             vlhs[:, dc2 * 128:(dc2 + 1) * 128], e,
                                             start=first, stop=last)
                        if first:
                            nc.vector.tensor_copy(out=den_v, in_=e)
                        else:
                            nc.vector.tensor_tensor(out=den_v, in0=den_v, in1=e,
                                                    op=ALU.add)
                    return num_ps, den_v

                def emit_post(qc, num_ps, den_v):
                    # numW psum -> sbuf (bf16) for the Wv application
                    numW = []
                    for dc in range(2):
                        t = wrk.tile([128, 512], bf16, tag=f"numW{dc}",
                                     name=f"numW{dc}_{qc}")
                        nc.vector.tensor_copy(out=t, in_=num_ps[dc])
                        numW.append(t)
                    # per-128q finalize partials: [128, 257] = Wv^T numW | 2*den
                    half, part = qc // 2, qc % 2
                    for qq_ in range(4):
                        fin = fps.tile([128, D + 1], f32, tag="fin",
                                       name=f"fin_{qc}_{qq_}")
                        for dc in range(2):
                            nc.tensor.matmul(fin[:, 0:D],
                                             numW[dc][:, qq_ * 128:(qq_ + 1) * 128],
                                             wvT[dc], start=(dc == 0), stop=(dc == 1))
                        nc.tensor.matmul(fin[:, D:D + 1],
                                         den_v[:, qq_ * 128:(qq_ + 1) * 128],
                                         onecol32, start=True, stop=True)
                        cp = wrk.tile([128, D + 1], bf16, tag="fincp",
                                      name=f"fincp_{qc}_{qq_}")
                        nc.scalar.copy(out=cp[:, 0:D], in_=fin[:, 0:D])
                        nc.scalar.mul(out=cp[:, D:D + 1], in_=fin[:, D:D + 1], mul=2.0)
                        base = part * 512 + qq_ * 128
                        nc.sync.dma_start(out=rs_in[half][base:base + 64, :],
                                          in_=cp[0:64, :])
                        nc.sync.dma_start(out=rs_in[half][base + 64:base + 128, :],
                                          in_=cp[64:128, :])
                    if part == 1:
                        nc.gpsimd.collective_compute(
                            "ReduceScatter", ALU.add,
                            replica_groups=[list(range(NC))],
                            ins=[rs_in[half][:, :].opt()],
                            outs=[rs_out[half][:, :].opt()],
                        )

                # software pipeline: chunk qc's post-processing is emitted
                # after chunk qc+1's flash loop so its drain overlaps compute
                state = {}
                for qc in range(4):
                    state[qc] = emit_loop(qc)
                    if qc >= 1:
                        with tc.high_priority():
                            emit_post(qc - 1, *state[qc - 1])
                with tc.high_priority():
                    emit_post(3, *state[3])

                # ---------------- finalize: two 128-row output halves --------
                # Pinned late in the simulated timeline so these (collective-
                # gated) ops land after all attention work in the engine queues.
                for h in range(2):
                    with tc.tile_wait_until(0.5 + 0.01 * h):
                        fo = per.tile([128, D + 1], bf16, tag=f"fo_{h}")
                        nc.sync.dma_start(out=fo[0:64, :], in_=rs_out[h][0:64, :])
                        nc.sync.dma_start(out=fo[64:128, :], in_=rs_out[h][64:128, :])
                        rec = per.tile([128, 1], f32, tag=f"rec_{h}")
                        nc.vector.reciprocal(out=rec, in_=fo[:, D:D + 1])
                        osb = per.tile([128, D], f32, tag=f"osb_{h}")
                        nc.vector.scalar_tensor_tensor(
                            out=osb, in0=fo[:, 0:D], scalar=rec, in1=zkadj[h],
                            op0=ALU.mult, op1=ALU.add)
                        nc.sync.dma_start(out=out_ext[h * 128:h * 128 + 64, :],
                                          in_=osb[0:64, :])
                        nc.sync.dma_start(out=out_ext[h * 128 + 64:(h + 1) * 128, :],
                                          in_=osb[64:128, :])

    nc.compile()
    return nc


_NC_CACHE: list = []


def _get_nc() -> bacc.Bacc:
    if not _NC_CACHE:
        _NC_CACHE.append(build())
    return _NC_CACHE[0]


def _pack_tiles(a: np.ndarray) -> np.ndarray:
    # [T*128, C] -> [128, T*C] partition-major pack
    t = a.shape[0] // 128
    return np.ascontiguousarray(
        a.reshape(t, 128, a.shape[1]).transpose(1, 0, 2).reshape(128, -1))


def _make_in_maps(inputs: dict) -> list[dict[str, np.ndarray]]:
    z = np.asarray(inputs["z"], dtype=np.float32)
    labels = np.asarray(inputs["labels"]).astype(np.int32).reshape(1, N)
    memory = np.asarray(inputs["memory"], dtype=np.float32)
    mw = np.asarray(inputs["memory_weights"], dtype=np.float32).reshape(-1)
    rmean = np.asarray(inputs["running_mean"], dtype=np.float32).reshape(1, D)
    rcov = np.asarray(inputs["running_cov"], dtype=np.float32)
    bq = np.asarray(inputs["bq"], dtype=np.float32).reshape(-1)
    bv = np.asarray(inputs["bv"], dtype=np.float32).reshape(1, D)
    ws = {nm: np.asarray(inputs[nm], dtype=np.float32) for nm in ("Wq", "Wk", "Wv")}

    # z pack with ones column per tile: [128, 16*257]
    zp = np.ones((16, 128, D + 1), np.float16)
    zp[:, :, 0:D] = z.reshape(16, 128, D).astype(np.float16)
    zp = np.ascontiguousarray(zp.transpose(1, 0, 2).reshape(128, ZW))

    wqp = _pack_tiles(ws["Wq"]).astype(np.float16)
    wkp = _pack_tiles(ws["Wk"]).astype(np.float16)
    # Wv^T packed, bf16: wvtp[p, c*D+j] = Wv[j, c*128+p]
    wvt = np.ascontiguousarray(ws["Wv"].T)
    wvtp = _pack_tiles(wvt).astype(ml_dtypes.bfloat16)
    # z^T packed: ztp[p, c*N+n] = z[n, c*128+p]
    ztp = _pack_tiles(np.ascontiguousarray(z.T)).astype(np.float16)

    in_maps = []
    for c in range(NC):
        aux = np.empty((128, AUXW), np.float32)
        aux[:, AUX_MW:AUX_MW + 128] = mw.reshape(128, 128)
        aux[:, AUX_WLOC:AUX_WLOC + JT] = mw[c * JL:(c + 1) * JL].reshape(JT, 128).T
        aux[:, AUX_RCOV:AUX_RCOV + 512] = _pack_tiles(rcov)
        aux[:, AUX_BQ] = bq[0:128]
        aux[:, AUX_BQ + 1] = bq[128:256]
        aux[:, AUX_RM] = rmean[0, 0:128]
        aux[:, AUX_RM + 1] = rmean[0, 128:256]
        zk = np.concatenate([z[c * 128:(c + 1) * 128],
                             z[1024 + c * 128:1024 + (c + 1) * 128]], axis=0)
        zkp = _pack_tiles(zk)
        mloc = memory[c * JL:(c + 1) * JL]
        in_maps.append({
            "zp": zp,
            "ztp": ztp,
            "zktp": _pack_tiles(np.ascontiguousarray(zk.T)).astype(np.float16),
            "zkbp": zkp.astype(ml_dtypes.bfloat16),
            "zkf": zkp,
            "mtp": _pack_tiles(np.ascontiguousarray(mloc.T)).astype(np.float16),
            "mbp": _pack_tiles(mloc).astype(ml_dtypes.bfloat16),
            "wqp": wqp, "wkp": wkp, "wvtp": wvtp,
            "aux": np.ascontiguousarray(aux),
            "labels": labels,
            "rmrow": rmean,
            "bvrow": bv,
        })
    return in_maps


def run(inputs: dict, trace: bool = False):
    nc = _get_nc()
    in_maps = _make_in_maps(inputs)
    res = run_bass_kernel_spmd(nc, in_maps, core_ids=list(range(NC)), trace=trace)
    out = np.empty((N, D), np.float32)
    for c in range(NC):
        oc = res.results[c]["out"]
        out[c * 128:(c + 1) * 128] = oc[0:128]
        out[1024 + c * 128:1024 + (c + 1) * 128] = oc[128:256]
    return out, res


def kernel(**inputs) -> np.ndarray:
    out, _ = run(inputs)
    return out
